# revision 4
# baseline (speedup 1.0000x reference)
"""Octree deconv + per-octree group norm + relu, 8 trn2 cores.

Correct-by-construction variant: gathers via vector-DGE indirect DMA
(128 rows / instruction — the HW contract), driven from a For_i loop to
keep program size small. Sharding: one octree per core (batch_id sorted),
padded to NCAP nodes; padded nodes gather an appended zero row.
"""

import sys

if "/opt/trn_rl_repo" not in sys.path:
    sys.path.insert(0, "/opt/trn_rl_repo")

import numpy as np

N_NODES = 300_000
K_TAPS = 27
CIN = 32
COUT = 32
G_GROUPS = 8
CG = CIN // G_GROUPS
B_OCT = 8
EPS = 1e-5

NCAP = 38_400
KBLK = [(g, 128 if g < 6 else 96) for g in range(7)]

PROFILE = False
LAST_EXEC_NS = None
_cache = {}


def _build(n_data, ncap):
    import concourse.bacc as bacc
    import concourse.bass as bass
    from concourse import mybir
    from concourse.tile import TileContext

    F32 = mybir.dt.float32
    I32 = mybir.dt.int32

    nc = bacc.Bacc(None, target_bir_lowering=False)
    data_t = nc.dram_tensor("data_t", [n_data + 1, CIN], F32, kind="ExternalInput")
    idx_t = nc.dram_tensor("idx_t", [ncap, K_TAPS], I32, kind="ExternalInput")
    wt_t = nc.dram_tensor("wt_t", [128, 7, COUT], F32, kind="ExternalInput")
    aux_t = nc.dram_tensor("aux_t", [COUT, 4], F32, kind="ExternalInput")
    gsel_t = nc.dram_tensor("gsel_t", [COUT, COUT], F32, kind="ExternalInput")
    ident_t = nc.dram_tensor("ident_t", [128, 128], F32, kind="ExternalInput")
    out_t = nc.dram_tensor("out_t", [COUT, ncap], F32, kind="ExternalOutput")

    with TileContext(nc) as tc:
        with (
            tc.tile_pool(name="const", bufs=1) as constp,
            tc.tile_pool(name="work", bufs=3) as workp,
            tc.tile_pool(name="ph2", bufs=2) as ph2p,
            tc.tile_pool(name="psxt", bufs=2, space="PSUM") as psxtp,
            tc.tile_pool(name="psh", bufs=2, space="PSUM") as pshp,
            tc.tile_pool(name="psg", bufs=1, space="PSUM") as psgp,
            tc.tile_pool(name="dram", bufs=1, space="DRAM") as dramp,
        ):
            wt = constp.tile([128, 7, COUT], F32)
            nc.sync.dma_start(out=wt[:], in_=wt_t[:])
            aux = constp.tile([COUT, 4], F32)
            nc.sync.dma_start(out=aux[:], in_=aux_t[:])
            gsel = constp.tile([COUT, COUT], F32)
            nc.sync.dma_start(out=gsel[:], in_=gsel_t[:])
            ident = constp.tile([128, 128], F32)
            nc.sync.dma_start(out=ident[:], in_=ident_t[:])
            eps_c = constp.tile([COUT, 1], F32)
            nc.vector.memset(eps_c[:], EPS)
            acc1 = constp.tile([COUT, 1], F32)
            nc.vector.memset(acc1[:], 0.0)
            acc2 = constp.tile([COUT, 1], F32)
            nc.vector.memset(acc2[:], 0.0)
            ht = dramp.tile([COUT, ncap], F32)

            def body(i):
                idx = workp.tile([128, K_TAPS], I32, tag="idx")
                nc.sync.dma_start(out=idx[:], in_=idx_t[bass.ds(i, 128), :])
                gsub = workp.tile([128, K_TAPS, CIN], F32, tag="gsub")
                for k in range(K_TAPS):
                    nc.gpsimd.indirect_dma_start(
                        out=gsub[:, k, :],
                        out_offset=None,
                        in_=data_t[:],
                        in_offset=bass.IndirectOffsetOnAxis(
                            ap=idx[:, k:k + 1], axis=0
                        ),
                    )
                xflat = gsub[:].rearrange("p a b -> p (a b)")
                ps_xt = psxtp.tile([128, 7, 128], F32, tag="ps_xt")
                for g, kg in KBLK:
                    nc.tensor.transpose(
                        out=ps_xt[0:kg, g, :],
                        in_=xflat[:, g * 128:g * 128 + kg],
                        identity=ident[:],
                    )
                xt = workp.tile([128, 7, 128], F32, tag="xt")
                nc.vector.tensor_copy(out=xt[:, 0:6, :], in_=ps_xt[:, 0:6, :])
                nc.vector.tensor_copy(out=xt[0:96, 6, :], in_=ps_xt[0:96, 6, :])
                ps_h = pshp.tile([COUT, 128], F32, tag="ps_h")
                for g, kg in KBLK:
                    nc.tensor.matmul(
                        out=ps_h[:],
                        lhsT=wt[0:kg, g, :],
                        rhs=xt[0:kg, g, :],
                        start=(g == 0),
                        stop=(g == 6),
                    )
                s1 = workp.tile([COUT, 1], F32, tag="s1")
                nc.vector.tensor_reduce(
                    out=s1[:], in_=ps_h[:], axis=mybir.AxisListType.X,
                    op=mybir.AluOpType.add,
                )
                nc.vector.tensor_add(acc1[:], acc1[:], s1[:])
                h2 = workp.tile([COUT, 128], F32, tag="h2")
                nc.scalar.square(out=h2[:], in_=ps_h[:])
                s2 = workp.tile([COUT, 1], F32, tag="s2")
                nc.vector.tensor_reduce(
                    out=s2[:], in_=h2[:], axis=mybir.AxisListType.X,
                    op=mybir.AluOpType.add,
                )
                nc.vector.tensor_add(acc2[:], acc2[:], s2[:])
                h_sb = workp.tile([COUT, 128], F32, tag="h_sb")
                nc.scalar.copy(out=h_sb[:], in_=ps_h[:])
                nc.sync.dma_start(out=ht[:, bass.ds(i, 128)], in_=h_sb[:])

            tc.For_i_unrolled(0, ncap, 128, body, max_unroll=8)

            # ---- group-norm coefficients -------------------------------
            stot = workp.tile([COUT, 2], F32, tag="stot")
            nc.vector.tensor_copy(out=stot[:, 0:1], in_=acc1[:])
            nc.vector.tensor_copy(out=stot[:, 1:2], in_=acc2[:])
            ps_gs = psgp.tile([COUT, 2], F32)
            nc.tensor.matmul(out=ps_gs[:], lhsT=gsel[:], rhs=stot[:],
                             start=True, stop=True)
            gsb = workp.tile([COUT, 2], F32, tag="gsb")
            nc.vector.tensor_scalar(
                out=gsb[:], in0=ps_gs[:], scalar1=aux[:, 2:3], scalar2=None,
                op0=mybir.AluOpType.mult,
            )
            var = workp.tile([COUT, 1], F32, tag="var")
            nc.vector.tensor_mul(var[:], gsb[:, 0:1], gsb[:, 0:1])
            nc.vector.tensor_sub(var[:], gsb[:, 1:2], var[:])
            std = workp.tile([COUT, 1], F32, tag="std")
            nc.scalar.activation(
                out=std[:], in_=var[:],
                func=mybir.ActivationFunctionType.Sqrt,
                bias=eps_c[:], scale=1.0,
            )
            istd = workp.tile([COUT, 1], F32, tag="istd")
            nc.vector.reciprocal(istd[:], std[:])
            coefa = workp.tile([COUT, 1], F32, tag="coefa")
            nc.vector.tensor_mul(coefa[:], istd[:], aux[:, 0:1])
            coefb = workp.tile([COUT, 1], F32, tag="coefb")
            nc.vector.tensor_mul(coefb[:], gsb[:, 0:1], coefa[:])
            nc.vector.tensor_sub(coefb[:], aux[:, 1:2], coefb[:])

            # ---- phase 2: normalize + relu -----------------------------
            PW = 1920

            def body2(j):
                hin = ph2p.tile([COUT, PW], F32, tag="hin")
                nc.sync.dma_start(out=hin[:], in_=ht[:, bass.ds(j, PW)])
                o = ph2p.tile([COUT, PW], F32, tag="o")
                nc.scalar.activation(
                    out=o[:], in_=hin[:],
                    func=mybir.ActivationFunctionType.Relu,
                    bias=coefb[:], scale=coefa[:],
                )
                nc.sync.dma_start(out=out_t[:, bass.ds(j, PW)], in_=o[:])

            tc.For_i_unrolled(0, ncap, PW, body2, max_unroll=2)

    nc.finalize()
    return nc


def _host_prep(data, weights, gamma, beta, neigh, batch_id, n_data, ncap):
    bounds = np.searchsorted(np.asarray(batch_id), np.arange(B_OCT + 1))
    data_pad = np.zeros((n_data + 1, CIN), dtype=np.float32)
    data_pad[:n_data] = np.asarray(data, dtype=np.float32)
    wt_host = np.zeros((128, 7, COUT), dtype=np.float32)
    w = np.asarray(weights, dtype=np.float32)
    for g, kg in KBLK:
        for j in range(kg // 32):
            wt_host[j * 32:(j + 1) * 32, g, :] = w[4 * g + j]
    gsel_host = np.zeros((COUT, COUT), dtype=np.float32)
    for c in range(COUT):
        g0 = (c // CG) * CG
        gsel_host[c, g0:g0 + CG] = 1.0
    ident_host = np.eye(128, dtype=np.float32)
    neigh = np.asarray(neigh)
    in_maps = []
    for b in range(B_OCT):
        s, e = int(bounds[b]), int(bounds[b + 1])
        cnt = e - s
        if cnt > ncap:
            raise ValueError(f"octree {b}: {cnt} nodes > {ncap}")
        idx_host = np.full((ncap, K_TAPS), n_data, dtype=np.int32)
        idx_host[:cnt] = neigh[s:e]
        aux_host = np.zeros((COUT, 4), dtype=np.float32)
        aux_host[:, 0] = np.asarray(gamma, dtype=np.float32)
        aux_host[:, 1] = np.asarray(beta, dtype=np.float32)
        aux_host[:, 2] = np.float32(1.0 / (cnt * (CIN / G_GROUPS) + EPS))
        in_maps.append(dict(data_t=data_pad, idx_t=idx_host, wt_t=wt_host,
                            aux_t=aux_host, gsel_t=gsel_host,
                            ident_t=ident_host))
    return in_maps, bounds


def kernel(data, weights, gamma, beta, neigh, batch_id, n_batch=None):
    global LAST_EXEC_NS
    from concourse.bass_utils import run_bass_kernel_spmd

    key = (N_NODES, NCAP)
    if key not in _cache:
        _cache[key] = _build(N_NODES, NCAP)
    nc = _cache[key]
    in_maps, bounds = _host_prep(
        data, weights, gamma, beta, neigh, batch_id, N_NODES, NCAP
    )
    res = run_bass_kernel_spmd(nc, in_maps, core_ids=list(range(B_OCT)),
                               trace=PROFILE)
    LAST_EXEC_NS = res.exec_time_ns
    out = np.empty((N_NODES, COUT), dtype=np.float32)
    for b in range(B_OCT):
        s, e = int(bounds[b]), int(bounds[b + 1])
        out[s:e] = res.results[b]["out_t"][:, : e - s].T
    return out
